# revision 8
# baseline (speedup 1.0000x reference)
# Trainium2 Bass kernel for nn_Attention_15917148799490 (sparse pooled attention block).
# Data-parallel over batch: 16 images -> 8 NeuronCores, 2 images per core.
# On-chip layout is channel-major ([C, H*W]); LayerNorm runs token-major and the
# activations are transposed once on the TensorEngine. The depthwise 7x7 convs run
# on the vector engine as 49 fused scalar_tensor_tensor FMA taps over a zero-padded
# [C, 62, 62] image. The 7x7->56x56 bilinear resize is a dense [49, 3136] matmul
# with a host-precomputed interpolation matrix.
from contextlib import ExitStack

import numpy as np
import ml_dtypes

import concourse.bass as bass
import concourse.mybir as mybir
import concourse.tile as tile
from concourse import bacc
from concourse.bass_utils import run_bass_kernel_spmd

BF = mybir.dt.bfloat16
F32 = mybir.dt.float32
AF = mybir.ActivationFunctionType
ALU = mybir.AluOpType
AX = mybir.AxisListType

B, H, W, C = 16, 56, 56, 384
C2, NH, WIN, HD2 = 192, 8, 7, 24
SCALE = HD2 ** -0.5
T = H * W          # 3136
N_CORES = 8
IPC = B // N_CORES  # images per core

N512 = [(i * 512, 512) for i in range(6)] + [(3072, 64)]
N448 = [(i * 448, 448) for i in range(7)]
TCH = [(i * 128, 128) for i in range(24)] + [(3072, 64)]
KS7 = [128, 128, 128, 128, 64, 64, 128]

bf16 = ml_dtypes.bfloat16

# bias column offsets in the packed bias matrix
BQ, BQC, BL, BKV, BA, BEF, BEB, BSCL, BP, BPE = 0, 3, 5, 8, 14, 17, 19, 21, 24, 27
NBIAS = 29


def _resize_mat(out_n, in_n):
    # jax.image.resize(method='bilinear'): sample at (o+0.5)*in/out - 0.5,
    # triangle kernel with edge clamp.
    R = np.zeros((out_n, in_n), np.float64)
    s = in_n / out_n
    for o in range(out_n):
        x = (o + 0.5) * s - 0.5
        x0 = int(np.floor(x))
        w1 = x - x0
        for idx, wt in [(x0, 1 - w1), (x0 + 1, w1)]:
            R[o, min(max(idx, 0), in_n - 1)] += wt
    return R.astype(np.float32)


class Ctx:
    pass


def _declare(nc):
    g = Ctx()
    dp = nc.declare_dram_parameter
    g.x_d = dp("x", [IPC * T, C], F32, isOutput=False)
    g.xe_d = dp("xe", [IPC * T, C2], F32, isOutput=False)
    g.wq_d = dp("wq", [3, 128, C], BF, isOutput=False)
    g.wqcut_d = dp("wqcut", [3, 128, C2], BF, isOutput=False)
    g.wl_d = dp("wl", [3, 128, C], BF, isOutput=False)
    g.wkv_d = dp("wkv", [3, 128, 512], BF, isOutput=False)
    g.wa_d = dp("wa", [3, 128, C], BF, isOutput=False)
    g.wef_d = dp("wef", [2, 128, C2], BF, isOutput=False)
    g.web_d = dp("web", [2, 128, C2], BF, isOutput=False)
    g.wscl_d = dp("wscl", [5, 128, 256], BF, isOutput=False)
    g.wproj_d = dp("wproj", [7, 128, C], BF, isOutput=False)
    g.wproje_d = dp("wproje", [7, 128, C2], BF, isOutput=False)
    g.wconv_d = dp("wconv", [3, 128, 49], F32, isOutput=False)
    g.weconv_d = dp("weconv", [2, 128, 49], F32, isOutput=False)
    g.mres_d = dp("mres", [49, T], BF, isOutput=False)
    g.ident_d = dp("ident", [128, 128], BF, isOutput=False)
    g.ident32_d = dp("ident32", [128, 24], BF, isOutput=False)
    g.biases_d = dp("biases", [128, NBIAS], F32, isOutput=False)
    g.outx_d = dp("out_x", [IPC * T, C], F32, isOutput=True)
    g.outxe_d = dp("out_xe", [IPC * T, C2], F32, isOutput=True)
    return g


def _pools(es, tc):
    g = Ctx()
    P = tc.tile_pool
    g.consts = es.enter_context(P(name="consts", bufs=1))
    g.wpool = es.enter_context(P(name="wpool", bufs=1))
    g.tok = es.enter_context(P(name="tok", bufs=2))
    g.stat = es.enter_context(P(name="stat", bufs=3))
    g.Xp = es.enter_context(P(name="Xp", bufs=1))
    g.XEp = es.enter_context(P(name="XEp", bufs=1))
    g.padp = es.enter_context(P(name="pad", bufs=1))
    g.accp = es.enter_context(P(name="accp", bufs=1))
    g.kvp = es.enter_context(P(name="kvp", bufs=1))
    g.attn_s = es.enter_context(P(name="attn_s", bufs=1))
    g.pexpp = es.enter_context(P(name="pexp", bufs=2))
    g.ptp = es.enter_context(P(name="ptp", bufs=1))
    g.vtp = es.enter_context(P(name="vtp", bufs=1))
    g.ktp = es.enter_context(P(name="ktile", bufs=1))
    g.mtmp = es.enter_context(P(name="mtmp", bufs=1))
    g.ostage = es.enter_context(P(name="ostage", bufs=2))
    g.psA = es.enter_context(P(name="psA", bufs=2, space="PSUM"))
    g.psMM = es.enter_context(P(name="psMM", bufs=3, space="PSUM"))
    g.psL = es.enter_context(P(name="psL", bufs=2, space="PSUM"))
    g.psPV = es.enter_context(P(name="psPV", bufs=1, space="PSUM"))
    return g


def _load_consts(nc, g, p):
    c = Ctx()
    c.ident = p.consts.tile([128, 128], BF, tag="ident", name="ident")
    nc.sync.dma_start(c.ident[:], g.ident_d[:])
    c.ident32 = p.consts.tile([128, 24], BF, tag="ident32", name="ident32")
    nc.sync.dma_start(c.ident32[:], g.ident32_d[:])
    c.mres = p.consts.tile([49, T], BF, tag="mres", name="mres")
    nc.sync.dma_start(c.mres[:], g.mres_d[:])
    c.biases = p.consts.tile([128, NBIAS], F32, tag="biases", name="biases")
    nc.sync.dma_start(c.biases[:], g.biases_d[:])
    c.eps_t = p.consts.tile([128, 1], F32, tag="eps", name="eps")
    nc.vector.memset(c.eps_t[:], 1e-6)

    def wload(dram, nk, m, dt=BF):
        t = p.wpool.tile([128, nk, m], dt, tag=dram.name)
        nc.sync.dma_start(t[:], dram[:].rearrange("k p m -> p k m"))
        return t

    c.wq = wload(g.wq_d, 3, C)
    c.wqcut = wload(g.wqcut_d, 3, C2)
    c.wl = wload(g.wl_d, 3, C)
    c.wkv = wload(g.wkv_d, 3, 512)
    c.wa = wload(g.wa_d, 3, C)
    c.wef = wload(g.wef_d, 2, C2)
    c.web = wload(g.web_d, 2, C2)
    c.wscl = wload(g.wscl_d, 5, 256)
    c.wproj = wload(g.wproj_d, 7, C)
    c.wproje = wload(g.wproje_d, 7, C2)
    c.wconv = wload(g.wconv_d, 3, 49, F32)
    c.weconv = wload(g.weconv_d, 2, 49, F32)
    return c


def _ln_tile(nc, p, c, dram, row0, tsz, nch, xtag, htag, stag):
    xt = p.tok.tile([128, nch], F32, tag=xtag)
    nc.sync.dma_start(xt[:tsz], dram[row0: row0 + tsz, :])
    st6 = p.stat.tile([128, 6], F32, tag=stag + "6")
    nc.vector.bn_stats(st6[:tsz], xt[:tsz])
    mv = p.stat.tile([128, 2], F32, tag=stag + "mv")
    nc.vector.bn_aggr(mv[:tsz], st6[:tsz])
    rstd = p.stat.tile([128, 1], F32, tag=stag + "r")
    nc.scalar.activation(rstd[:tsz], mv[:tsz, 1:2], AF.Sqrt, bias=c.eps_t[:tsz], scale=1.0)
    nc.vector.reciprocal(rstd[:tsz], rstd[:tsz])
    xh = p.tok.tile([128, nch], BF, tag=htag)
    nc.vector.tensor_scalar(xh[:tsz], xt[:tsz], mv[:tsz, 0:1], rstd[:tsz, 0:1],
                            op0=ALU.subtract, op1=ALU.mult)
    return xh


def _phase_ln(nc, p, c, g, im):
    X = [p.Xp.tile([128, T], BF, tag=f"X{cc}", name=f"X{cc}") for cc in range(3)]
    XE = [p.XEp.tile([128, T], BF, tag=f"XE{cc}", name=f"XE{cc}") for cc in range(2)]
    for (t0, tsz) in TCH:
        xh = _ln_tile(nc, p, c, g.x_d, im * T + t0, tsz, C, "xt", "xh", "s")
        for cc in range(3):
            pst = p.psA.tile([128, 128], BF, tag="psA", name="psA")
            nc.tensor.transpose(pst[:, :tsz], xh[:tsz, cc * 128:(cc + 1) * 128],
                                c.ident[:tsz, :tsz])
            nc.scalar.copy(X[cc][:, t0:t0 + tsz], pst[:, :tsz])
        xeh = _ln_tile(nc, p, c, g.xe_d, im * T + t0, tsz, C2, "xet", "xeh", "se")
        pst = p.psA.tile([128, 128], BF, tag="psA", name="psA")
        nc.tensor.transpose(pst[:, :tsz], xeh[:tsz, 0:128], c.ident[:tsz, :tsz])
        nc.scalar.copy(XE[0][:, t0:t0 + tsz], pst[:, :tsz])
        pst = p.psA.tile([128, 128], BF, tag="psA", name="psA")
        nc.tensor.transpose(pst[:64, :tsz], xeh[:tsz, 128:192], c.ident[:tsz, :tsz])
        nc.scalar.copy(XE[1][:64, t0:t0 + tsz], pst[:64, :tsz])
    return X, XE


def _phase_pads(nc, p, c, X, XE):
    lxpad = [p.padp.tile([128, 62, 62], BF, tag=f"pad{k}", name=f"pad{k}") for k in range(3)]
    for k in range(3):
        nc.gpsimd.memset(lxpad[k][:], 0.0)
    for mi in range(3):
        for (n0, nsz) in N448:
            ps = p.psMM.tile([128, 512], F32, tag="psMM", name="psMM")
            for kc in range(3):
                nc.tensor.matmul(ps[:, :nsz], c.wl[:, kc, mi * 128:(mi + 1) * 128],
                                 X[kc][:, n0:n0 + nsz], start=(kc == 0), stop=(kc == 2))
            h0 = n0 // 56
            nc.scalar.activation(lxpad[mi][:, 3 + h0:3 + h0 + 8, 3:59],
                                 ps[:, :nsz].rearrange("p (a b) -> p a b", a=8),
                                 AF.Gelu, bias=c.biases[:, BL + mi:BL + mi + 1], scale=1.0)
    efpad = [p.padp.tile([128, 62, 62], BF, tag=f"pad{3 + k}", name=f"pad{3 + k}") for k in range(2)]
    for k in range(2):
        nc.gpsimd.memset(efpad[k][:], 0.0)
    for mi, msz in [(0, 128), (1, 64)]:
        for (n0, nsz) in N448:
            ps = p.psMM.tile([128, 512], F32, tag="psMM", name="psMM")
            for kc, ksz in [(0, 128), (1, 64)]:
                nc.tensor.matmul(ps[:msz, :nsz], c.wef[:ksz, kc, mi * 128:mi * 128 + msz],
                                 XE[kc][:ksz, n0:n0 + nsz], start=(kc == 0), stop=(kc == 1))
            h0 = n0 // 56
            nc.scalar.activation(efpad[mi][:msz, 3 + h0:3 + h0 + 8, 3:59],
                                 ps[:msz, :nsz].rearrange("p (a b) -> p a b", a=8),
                                 AF.Identity, bias=c.biases[:msz, BEF + mi:BEF + mi + 1],
                                 scale=1.0)
    return lxpad, efpad


def _phase_kv(nc, p, c, lxpad):
    # heads padded to a 32-partition stride: tile j holds heads 3j..3j+2 at
    # partition offsets 0/32/64 (matmul base-partition constraint)
    KH = [p.kvp.tile([88 if j < 2 else 56, T], BF, tag=f"K{j}", name=f"K{j}")
          for j in range(3)]
    VH = [p.kvp.tile([88 if j < 2 else 56, T], BF, tag=f"V{j}", name=f"V{j}")
          for j in range(3)]
    MB = [(0, 88), (96, 88), (192, 56), (256, 88), (352, 88), (448, 56)]
    for mi, (mb, msz) in enumerate(MB):
        dst = (KH + VH)[mi]
        for (n0, nsz) in N448:
            h0 = n0 // 56
            ps = p.psMM.tile([128, 512], F32, tag="psMM", name="psMM")
            for kc in range(3):
                nc.tensor.matmul(ps[:msz, :nsz], c.wkv[:, kc, mb:mb + msz],
                                 lxpad[kc][:, 3 + h0:3 + h0 + 8, 3:59],
                                 start=(kc == 0), stop=(kc == 2))
            nc.scalar.activation(dst[:msz, n0:n0 + nsz], ps[:msz, :nsz], AF.Identity,
                                 bias=c.biases[:msz, BKV + mi:BKV + mi + 1], scale=1.0)
    return KH, VH


def _phase_scl(nc, p, c, X, XE):
    pooledb = []
    for i, (src, ksz) in enumerate([(X[0], 128), (X[1], 128), (X[2], 128),
                                    (XE[0], 128), (XE[1], 64)]):
        s1 = p.stat.tile([128, 392], F32, tag="pool1", name="pool1")
        nc.vector.tensor_reduce(
            s1[:ksz], src[:ksz].rearrange("p (h wb ww) -> p h wb ww", h=56, wb=7),
            axis=AX.X, op=ALU.add)
        s2 = p.stat.tile([128, 49], F32, tag="pool2", name="pool2")
        nc.vector.tensor_reduce(
            s2[:ksz], s1[:ksz].rearrange("p (hb hh wb) -> p hb wb hh", hb=7, hh=8),
            axis=AX.X, op=ALU.add)
        pb = p.stat.tile([128, 49], BF, tag=f"pool3_{i}", name=f"pool3_{i}")
        nc.scalar.copy(pb[:ksz], s2[:ksz])
        pooledb.append((pb, ksz))
    Mq = [p.attn_s.tile([88 if j < 2 else 56, 49], BF, tag=f"Mq{j}", name=f"Mq{j}")
          for j in range(3)]
    for mi, (mb, msz) in enumerate([(0, 88), (96, 88), (192, 56)]):
        ps = p.psMM.tile([128, 512], F32, tag="psMM", name="psMM")
        for kc, (pb, ksz) in enumerate(pooledb):
            nc.tensor.matmul(ps[:msz, :49], c.wscl[:ksz, kc, mb:mb + msz],
                             pb[:ksz, :49], start=(kc == 0), stop=(kc == 4))
        nc.scalar.activation(Mq[mi][:msz, :], ps[:msz, :49], AF.Identity,
                             bias=c.biases[:msz, BSCL + mi:BSCL + mi + 1], scale=1.0)
    return Mq


def _dwconv(nc, inpad, acc, wv, npart):
    for t in range(49):
        dy, dx = t // 7, t % 7
        src = inpad[:npart, dy:dy + 56, dx:dx + 56]
        dst = acc[:npart].rearrange("p (a b) -> p a b", a=56)
        if t == 0:
            nc.vector.tensor_scalar(dst, src, wv[:npart, 0:1], None, op0=ALU.mult)
        else:
            nc.vector.scalar_tensor_tensor(dst, src, wv[:npart, t:t + 1], dst,
                                           op0=ALU.mult, op1=ALU.add)


def _phase_conv(nc, p, c, lxpad, efpad):
    cacc = [p.accp.tile([128, T], BF, tag=f"acc{k}", name=f"acc{k}") for k in range(3)]
    for k in range(3):
        _dwconv(nc, lxpad[k], cacc[k], c.wconv[:, k, :], 128)
    eacc = [p.accp.tile([128, T], BF, tag=f"acc{3 + k}", name=f"acc{3 + k}") for k in range(2)]
    _dwconv(nc, efpad[0], eacc[0], c.weconv[:, 0, :], 128)
    _dwconv(nc, efpad[1], eacc[1], c.weconv[:, 1, :], 64)
    return cacc, eacc


def _phase_attn(nc, p, c, Mq, KH, VH):
    O = p.attn_s.tile([49, C2], BF, tag="O", name="O")
    for h in range(NH):
        hb, ho = h // 3, (h % 3) * 32
        mq_h = Mq[hb][ho:ho + 24, :]
        k_h = KH[hb][ho:ho + 24, :]
        v_h = VH[hb][ho:ho + 24, :]
        sums = p.stat.tile([49, 7], F32, tag="sums", name="sums")
        pT = p.ptp.tile([128, 25, 49], BF, tag="pT", name="pT")
        vTt = p.vtp.tile([128, 25, 24], BF, tag="vT", name="vT")
        for ci, (n0, nsz) in enumerate(N512):
            psl = p.psL.tile([49, 512], F32, tag="psL", name="psL")
            nc.tensor.matmul(psl[:, :nsz], mq_h, k_h[:, n0:n0 + nsz], start=True, stop=True)
            pexp = p.pexpp.tile([49, 512], BF, tag="pexp", name="pexp")
            nc.scalar.activation(pexp[:, :nsz], psl[:, :nsz], AF.Exp, bias=0.0, scale=1.0,
                                 accum_out=sums[:, ci:ci + 1])
            for s in range(0, nsz, 128):
                tsz = min(128, nsz - s)
                tci = (n0 + s) // 128
                pst = p.psA.tile([128, 128], BF, tag="psA", name="psA")
                nc.tensor.transpose(pst[:tsz, :49], pexp[:49, s:s + tsz], c.ident[:49, :49])
                nc.scalar.copy(pT[:tsz, tci, :], pst[:tsz, :49])
        for (t0, tsz) in TCH:
            tci = t0 // 128
            pst = p.psA.tile([128, 128], BF, tag="psA", name="psA")
            nc.tensor.transpose(pst[:tsz, :24], v_h[:, t0:t0 + tsz], c.ident32[ho:ho + 24, :])
            nc.scalar.copy(vTt[:tsz, tci, :], pst[:tsz, :24])
        rowsum = p.stat.tile([49, 1], F32, tag="rowsum", name="rowsum")
        nc.vector.reduce_sum(rowsum[:, :], sums[:, :], axis=AX.X)
        recip = p.stat.tile([49, 1], F32, tag="recip", name="recip")
        nc.vector.reciprocal(recip[:, :], rowsum[:, :])
        pspv = p.psPV.tile([49, 24], F32, tag="psPV", name="psPV")
        for tc_i, (t0, tsz) in enumerate(TCH):
            nc.tensor.matmul(pspv[:, :], pT[:tsz, tc_i, :], vTt[:tsz, tc_i, :],
                             start=(tc_i == 0), stop=(tc_i == 24))
        nc.scalar.activation(O[:, h * 24:(h + 1) * 24], pspv[:, :], AF.Copy,
                             bias=0.0, scale=recip[:, 0:1])
    return O


def _mm_epi(nc, p, c, w, rhs_chunks, mlo, msz, nsz, bcol, func, dsttag):
    """matmul over k-chunks + ACT epilogue into a fresh mtmp tile; returns it."""
    ps = p.psMM.tile([128, 512], F32, tag="psMM", name="psMM")
    nk = len(rhs_chunks)
    for kc, (rap, ksz) in enumerate(rhs_chunks):
        nc.tensor.matmul(ps[:msz, :nsz], w[:ksz, kc, mlo:mlo + msz], rap,
                         start=(kc == 0), stop=(kc == nk - 1))
    out = p.mtmp.tile([128, 512], BF, tag=dsttag)
    nc.scalar.activation(out[:msz, :nsz], ps[:msz, :nsz], func,
                         bias=c.biases[:msz, bcol:bcol + 1], scale=1.0)
    return out


def _phase_out(nc, p, c, g, im, X, cacc, eacc, O, n0, nsz):
    kt = [p.ktp.tile([128, 512], BF, tag=f"kt{i}", name=f"kt{i}") for i in range(7)]
    Xr = [(X[kc][:, n0:n0 + nsz], 128) for kc in range(3)]
    Ar = [(cacc[kc][:, n0:n0 + nsz], 128) for kc in range(3)]
    Er = [(eacc[0][:, n0:n0 + nsz], 128), (eacc[1][:64, n0:n0 + nsz], 64)]
    # g = q * a -> kt[0..2]
    for mi in range(3):
        qtmp = _mm_epi(nc, p, c, c.wq, Xr, mi * 128, 128, nsz, BQ + mi, AF.Identity, "qtmp")
        atmp = _mm_epi(nc, p, c, c.wa, Ar, mi * 128, 128, nsz, BA + mi, AF.Identity, "atmp")
        nc.vector.tensor_mul(kt[mi][:, :nsz], qtmp[:, :nsz], atmp[:, :nsz])
    # attn -> kt[3], kt[4] (rows 0:64)
    ps = p.psMM.tile([128, 512], F32, tag="psMM", name="psMM")
    nc.tensor.matmul(ps[:, :nsz], O[:, 0:128], c.mres[:, n0:n0 + nsz], start=True, stop=True)
    nc.scalar.copy(kt[3][:, :nsz], ps[:, :nsz])
    ps = p.psMM.tile([128, 512], F32, tag="psMM", name="psMM")
    nc.tensor.matmul(ps[:64, :nsz], O[:, 128:192], c.mres[:, n0:n0 + nsz],
                     start=True, stop=True)
    nc.scalar.copy(kt[4][:64, :nsz], ps[:64, :nsz])
    # cut = qcut * xe2 -> kt[5] rows 0:64, kt[6] rows 0:128
    for (mi, mlo, msz, kti) in [(0, 0, 64, 5), (1, 64, 128, 6)]:
        ctmp = _mm_epi(nc, p, c, c.wqcut, Xr, mlo, msz, nsz, BQC + mi, AF.Identity, "ctmp")
        xtmp = _mm_epi(nc, p, c, c.web, Er, mlo, msz, nsz, BEB + mi, AF.Identity, "xtmp")
        nc.vector.tensor_mul(kt[kti][:msz, :nsz], ctmp[:msz, :nsz], xtmp[:msz, :nsz])
    # proj / proje
    Kr = [(kt[kc][:KS7[kc], :nsz], KS7[kc]) for kc in range(7)]
    xotmp = [_mm_epi(nc, p, c, c.wproj, Kr, mi * 128, 128, nsz, BP + mi, AF.Identity,
                     f"xo{mi}") for mi in range(3)]
    xeotmp = [_mm_epi(nc, p, c, c.wproje, Kr, mi * 128, msz, nsz, BPE + mi, AF.Identity,
                      f"xeo{mi}") for mi, msz in [(0, 128), (1, 64)]]
    for s in range(0, nsz, 128):
        tsz = min(128, nsz - s)
        stg = p.ostage.tile([128, C], F32, tag="ostg", name="ostg")
        for mi in range(3):
            pst = p.psA.tile([128, 128], BF, tag="psA", name="psA")
            nc.tensor.transpose(pst[:tsz, :], xotmp[mi][:, s:s + tsz], c.ident[:, :])
            nc.scalar.copy(stg[:tsz, mi * 128:(mi + 1) * 128], pst[:tsz, :])
        nc.sync.dma_start(g.outx_d[im * T + n0 + s: im * T + n0 + s + tsz, :], stg[:tsz])
        stge = p.ostage.tile([128, C2], F32, tag="ostge", name="ostge")
        pst = p.psA.tile([128, 128], BF, tag="psA", name="psA")
        nc.tensor.transpose(pst[:tsz, :], xeotmp[0][:, s:s + tsz], c.ident[:, :])
        nc.scalar.copy(stge[:tsz, 0:128], pst[:tsz, :])
        pst = p.psA.tile([128, 128], BF, tag="psA", name="psA")
        nc.tensor.transpose(pst[:tsz, :64], xeotmp[1][:64, s:s + tsz], c.ident[:64, :64])
        nc.scalar.copy(stge[:tsz, 128:192], pst[:tsz, :64])
        nc.sync.dma_start(g.outxe_d[im * T + n0 + s: im * T + n0 + s + tsz, :], stge[:tsz])


def build_nc():
    nc = bacc.Bacc(None, target_bir_lowering=False)
    g = _declare(nc)
    with tile.TileContext(nc) as tc, ExitStack() as es:
        p = _pools(es, tc)
        c = _load_consts(nc, g, p)
        for im in range(IPC):
            X, XE = _phase_ln(nc, p, c, g, im)
            lxpad, efpad = _phase_pads(nc, p, c, X, XE)
            KH, VH = _phase_kv(nc, p, c, lxpad)
            Mq = _phase_scl(nc, p, c, X, XE)
            cacc, eacc = _phase_conv(nc, p, c, lxpad, efpad)
            O = _phase_attn(nc, p, c, Mq, KH, VH)
            for (n0, nsz) in N512:
                _phase_out(nc, p, c, g, im, X, cacc, eacc, O, n0, nsz)
    nc.compile()
    return nc


_CACHE = {}


def _prep_inputs(inputs):
    ii = {k: np.asarray(v, np.float32) for k, v in inputs.items()}
    gx, bx = ii["norm_w"], ii["norm_b"]
    ge, be = ii["norme_w"], ii["norme_b"]

    def fold(wn, bn, gw, bb_):
        w, bb = ii[wn], ii[bn]
        return (gw[:, None] * w), (bb + bb_ @ w)

    q_w, q_b = fold("q_w", "q_b", gx, bx)
    qcut_w, qcut_b = fold("qcut_w", "qcut_b", gx, bx)
    l_w, l_b = fold("l_w", "l_b", gx, bx)
    efore_w, efore_b = fold("efore_w", "efore_b", ge, be)
    gcat = np.concatenate([gx, ge])
    bcat = np.concatenate([bx, be])
    scl_w = (gcat[:, None] * ii["scl_w"]) * (SCALE / 64.0)
    scl_b = (ii["scl_b"] + bcat @ ii["scl_w"]) * SCALE
    a_b = ii["a_b"] + ii["conv_b"] @ ii["a_w"]
    eback_b = ii["eback_b"] + ii["econv_b"] @ ii["eback_w"]

    def pad_heads(w):
        # [*, 192] head-major cols (h*24+d) -> [*, 256] padded (h*32+d)
        out = np.zeros((w.shape[0], 256), np.float32)
        for h in range(8):
            out[:, h * 32:h * 32 + 24] = w[:, h * 24:(h + 1) * 24]
        return out

    kv_pad = np.concatenate([pad_heads(ii["kv_w"][:, :192]),
                             pad_heads(ii["kv_w"][:, 192:])], axis=1)  # [384, 512]
    scl_pad = pad_heads(scl_w)                                          # [576, 256]

    def padvec(v):
        out = np.zeros(256, np.float32)
        for h in range(8):
            out[h * 32:h * 32 + 24] = v[h * 24:(h + 1) * 24]
        return out

    kvb_pad = np.concatenate([padvec(ii["kv_b"][:192]), padvec(ii["kv_b"][192:])])
    sclb_pad = padvec(scl_b)


    def chunks(w, bounds, m, dt=bf16):
        out = np.zeros((len(bounds), 128, m), np.float32)
        for k, (lo, hi) in enumerate(bounds):
            out[k, :hi - lo] = w[lo:hi]
        return out.astype(dt)

    def even(n, kin, step=128):
        return [(i * step, min((i + 1) * step, kin)) for i in range(n)]

    P7 = [(0, 128), (128, 256), (256, 384), (384, 512), (512, 576), (576, 640), (640, 768)]

    def biascols(vecs):
        cols = np.zeros((128, len(vecs)), np.float32)
        for i, v in enumerate(vecs):
            cols[:len(v), i] = v
        return cols

    bias_mat = np.concatenate([
        biascols([q_b[0:128], q_b[128:256], q_b[256:384]]),
        biascols([qcut_b[0:64], qcut_b[64:192]]),
        biascols([l_b[0:128], l_b[128:256], l_b[256:384]]),
        biascols([kvb_pad[0:88], kvb_pad[96:184], kvb_pad[192:248],
                  kvb_pad[256:344], kvb_pad[352:440], kvb_pad[448:504]]),
        biascols([a_b[0:128], a_b[128:256], a_b[256:384]]),
        biascols([efore_b[0:128], efore_b[128:192]]),
        biascols([eback_b[0:64], eback_b[64:192]]),
        biascols([sclb_pad[0:88], sclb_pad[96:184], sclb_pad[192:248]]),
        biascols([ii["proj_b"][0:128], ii["proj_b"][128:256], ii["proj_b"][256:384]]),
        biascols([ii["proje_b"][0:128], ii["proje_b"][128:192]]),
    ], axis=1)
    assert bias_mat.shape[1] == NBIAS

    Rh = _resize_mat(56, 7)
    mres = np.kron(Rh, Rh).T.astype(bf16).copy()  # [49, 3136]
    ident32 = np.zeros((128, 24), np.float32)
    for pp in range(128):
        if pp % 32 < 24:
            ident32[pp, pp % 32] = 1.0
    ident32 = ident32.astype(bf16)

    weights = dict(
        wq=chunks(q_w, even(3, C), C), wqcut=chunks(qcut_w, even(3, C), C2),
        wl=chunks(l_w, even(3, C), C), wkv=chunks(kv_pad, even(3, C), 512),
        wa=chunks(ii["a_w"], even(3, C), C),
        wef=chunks(efore_w, even(2, C2), C2), web=chunks(ii["eback_w"], even(2, C2), C2),
        wscl=chunks(scl_pad, even(5, 576), 256),
        wproj=chunks(ii["proj_w"], P7, C), wproje=chunks(ii["proje_w"], P7, C2),
        wconv=chunks(ii["conv_w"].reshape(C, 49), even(3, C), 49, np.float32),
        weconv=chunks(ii["econv_w"].reshape(C2, 49), even(2, C2), 49, np.float32),
        mres=mres, ident=np.eye(128, dtype=bf16), ident32=ident32,
        biases=bias_mat,
    )
    return weights


def kernel(**inputs):
    if "nc" not in _CACHE:
        _CACHE["nc"] = build_nc()
    nc = _CACHE["nc"]
    weights = _prep_inputs(inputs)
    x = np.ascontiguousarray(np.asarray(inputs["x"], np.float32).reshape(B, T, C))
    xe = np.ascontiguousarray(np.asarray(inputs["x_e"], np.float32).reshape(B, T, C2))
    in_maps = []
    for core in range(N_CORES):
        im = dict(weights)
        im["x"] = np.ascontiguousarray(x[core * IPC:(core + 1) * IPC].reshape(IPC * T, C))
        im["xe"] = np.ascontiguousarray(xe[core * IPC:(core + 1) * IPC].reshape(IPC * T, C2))
        in_maps.append(im)
    res = run_bass_kernel_spmd(nc, in_maps, list(range(N_CORES))).results
    xo = np.concatenate([res[cc]["out_x"].reshape(IPC, H, W, C) for cc in range(N_CORES)])
    xeo = np.concatenate([res[cc]["out_xe"].reshape(IPC, H, W, C2) for cc in range(N_CORES)])
    return xo, xeo
